# revision 1
# baseline (speedup 1.0000x reference)
"""APPNP forward on 8 Trainium2 NeuronCores — ap_gather design (v2).

Reference: h = features; 10x: h = 0.9 * (segment_sum((h*ns)[src] by dst) * nd)
+ 0.1 * h0.  Nodes sharded 8 ways (12544 rows/core); edges partitioned by dst
core.

Per step, per core:
  1. The global scaled table g = h*ns lives in SBUF transposed+quantized:
     partition 16g+j holds, for src-shard g, words (m, r) = g[shard_g node m,
     feat 16r+j] (bf16, d=4 elems/idx).  128 partitions = 8 shards x 16 rows.
  2. ap_gather (GPSIMD, 8 Q7 cores in parallel, one per src shard) fetches
     per-edge rows: each core's idx list = its shard's edges for one
     superchunk, ordered by (dst window, slot).  ~29ns/idx/core, 8 edges per
     idx-slot across the engine.
  3. Per 128-slot chunk: 4 PE matmuls (lhsT = strided gather-output view,
     rhs = I128) transpose all 8 groups at once into PSUM [slots, 512];
     one DVE copy -> SBUF msg bf16.
  4. Scatter: per (chunk, group, window-run) segment, one-hot built by DVE
     is_equal from a resident dstw table, then matmul(psum agg[w],
     lhsT=oh, rhs=msg[0:nk, r-strided group view]) accumulating by window.
  5. Blend (DVE) agg*a + b -> bf16; 4 PE transposes + 1 DVE copy produce the
     next table's 16-row strip; DMA per superchunk into Tin_T (DRAM).
  6. Split AllGather (2 column-range halves, non-Shared out) Tin_T ->
     T_T_all; 2 DMA loads refill the SBUF table.  The first half fires while
     late superchunks still gather, hiding most of the collective.

The schedule (section sizes per (window, src-shard) = max over cores,
segment structure, start/stop flags) is shared across cores; per-core data
(idx values, dstw columns) differ.  All static structure computed on host.
"""

import sys

sys.path.insert(0, "/opt/trn_rl_repo")

import numpy as np
import ml_dtypes

BF16 = ml_dtypes.bfloat16

K_LAYERS = 10
ALPHA = 0.1
N_NODES = 100_000
D_FEAT = 64
M_CORES = 8

W = 128      # dst window width (psum partitions)
SHARD = 12544
NW = SHARD // W          # 98 windows/core
G_WIN = 7                # windows per superchunk
NSC = NW // G_WIN        # 14 superchunks


def _preprocess(src, dst, M=M_CORES):
    E = src.shape[0]
    core = dst // SHARD
    ldst = dst - core * SHARD
    w = ldst // W
    dw = (ldst - w * W).astype(np.int64)
    g = src // SHARD
    m = (src - g * SHARD).astype(np.int64)
    sc = w // G_WIN

    # shared section sizes: n_wg[w, g] = max over cores of count
    key_cwg = (core * NW + w) * M + g
    cnt = np.bincount(key_cwg, minlength=M * NW * M).reshape(M, NW, M)
    n_wg = cnt.max(axis=0)            # [NW, G=8]
    n_wg[:, 0] = np.maximum(n_wg[:, 0], 1)   # every window gets >=1 slot

    # per (sc, g): stream length = sum of its windows' sections, round to 32
    sec_of = n_wg.reshape(NSC, G_WIN, M)     # [sc, wi, g]
    stream_len = sec_of.sum(axis=1)          # [sc, g]
    L_sc = ((stream_len.max(axis=1) + 31) // 32) * 32   # [sc]
    L_off = np.concatenate([[0], np.cumsum(L_sc)])
    L_tot = int(L_off[-1])

    # slot offset of each (sc, wi, g) section within its (sc, g) stream
    sec_start = np.zeros((NSC, G_WIN, M), dtype=np.int64)
    sec_start[:, 1:, :] = np.cumsum(sec_of, axis=1)[:, :-1, :]

    # place edges: rank within (core, w, g), slot = sec_start + rank
    order = np.argsort(key_cwg, kind="stable")
    starts = np.concatenate([[0], np.cumsum(np.bincount(key_cwg, minlength=M * NW * M))])
    rank = np.arange(E) - starts[key_cwg[order]]
    oc, ow, og, om, odw = core[order], w[order], g[order], m[order], dw[order]
    osc = ow // G_WIN
    owi = ow - osc * G_WIN
    slot = sec_start[osc, owi, og] + rank    # slot within (sc, g) stream

    # idx arrays [M, 128, L_tot/16] int16  (partition 16g+p, word t//16)
    idx16 = np.zeros((M, 128, L_tot // 16), dtype=np.int16)
    part = 16 * og + (slot % 16)
    word = L_off[osc] // 16 + slot // 16
    idx16[oc, part, word] = om.astype(np.int16)

    # schedule: per sc, per chunk k, segments (g, w_global, col, start, stop)
    # segment = (k, g, wi) incidence; cols indexed globally
    segs = []      # list per sc of list per k of list of (g, w, col)
    seg_dw_rows = []   # (col -> (sc, k, g, w, s0, s1)) for dstw fill
    col = 0
    win_seg_cols = [[] for _ in range(NW)]   # ordered segment cols per window
    for s in range(NSC):
        nch = int(L_sc[s]) // 128 + (1 if int(L_sc[s]) % 128 else 0)
        per_k = []
        for k in range(nch):
            k0, k1 = 128 * k, min(128 * (k + 1), int(L_sc[s]))
            entry = []
            for gg in range(M):
                for wi in range(G_WIN):
                    s0 = int(sec_start[s, wi, gg])
                    s1 = s0 + int(sec_of[s, wi, gg])
                    a, b = max(s0, k0), min(s1, k1)
                    if a < b:
                        wglob = s * G_WIN + wi
                        entry.append((gg, wglob, col, a - k0, b - k0))
                        win_seg_cols[wglob].append(col)
                        col += 1
            per_k.append((k0, k1, entry))
        segs.append(per_k)
    nseg_tot = col
    max_nseg_k = max(len(e) for per_k in segs for _, _, e in per_k)

    # start/stop flags per segment col
    startf = np.zeros(nseg_tot, dtype=bool)
    stopf = np.zeros(nseg_tot, dtype=bool)
    for wglob in range(NW):
        cols = win_seg_cols[wglob]
        assert cols, wglob
        startf[cols[0]] = True
        stopf[cols[-1]] = True

    # dstw data [M, 128, nseg_tot] bf16: slot p of chunk k in segment col
    dstw = np.full((M, 128, nseg_tot), -1.0, dtype=np.float32)
    max_chunks = max(len(per_k) for per_k in segs)
    colarr = np.full((NSC, max_chunks, M, G_WIN), -1, dtype=np.int64)
    for s in range(NSC):
        for (k0, k1, entry) in segs[s]:
            for (gg, wglob, c_, a, b) in entry:
                colarr[s, k0 // 128, gg, wglob - s * G_WIN] = c_
    ok = (slot // 128).astype(np.int64)
    ccols = colarr[osc, ok, og, owi]
    assert (ccols >= 0).all()
    dstw[oc, slot % 128, ccols] = odw
    dstw = dstw.astype(BF16)

    return dict(
        L_sc=L_sc, L_off=L_off, L_tot=L_tot, segs=segs,
        startf=startf, stopf=stopf, nseg_tot=nseg_tot,
        max_nseg_k=max_nseg_k, idx16=idx16, dstw=dstw,
    )


def _build_nc(meta, M, D, steps, agsplit):
    from concourse import bass, bacc, tile, mybir

    dt = mybir.dt
    L_sc, L_off, L_tot = meta["L_sc"], meta["L_off"], meta["L_tot"]
    segs, startf, stopf = meta["segs"], meta["startf"], meta["stopf"]
    nseg_tot, max_nseg_k = meta["nseg_tot"], meta["max_nseg_k"]
    NT = SHARD * 4                      # table cols per partition (elems)
    Lmax = int(max(L_sc))

    nc = bacc.Bacc("TRN2", target_bir_lowering=False, debug=False, num_devices=M)

    tt0 = nc.dram_tensor("tt0", [128, NT], dt.bfloat16, kind="ExternalInput").ap()
    idx_d = nc.dram_tensor("idx", [128, L_tot // 16], dt.int16,
                           kind="ExternalInput").ap()
    dstw_d = nc.dram_tensor("dstw", [128, nseg_tot], dt.bfloat16,
                            kind="ExternalInput").ap()
    iota_d = nc.dram_tensor("iota", [128, W], dt.bfloat16, kind="ExternalInput").ap()
    ident_d = nc.dram_tensor("ident", [128, 128], dt.bfloat16,
                             kind="ExternalInput").ap()
    a_d = nc.dram_tensor("acoef", [W, NW], dt.float32, kind="ExternalInput").ap()
    b_d = nc.dram_tensor("bcoef", [W, NW * D], dt.bfloat16, kind="ExternalInput").ap()
    a2_d = nc.dram_tensor("acoef2", [W, NW], dt.float32, kind="ExternalInput").ap()
    b2_d = nc.dram_tensor("bcoef2", [W, NW * D], dt.bfloat16,
                          kind="ExternalInput").ap()
    out = nc.dram_tensor("out", [SHARD, D], dt.float32, kind="ExternalOutput").ap()
    import os
    dbg = int(os.environ.get("APPNP_DEBUG", "0"))
    if dbg:
        dbg_gout = nc.dram_tensor("dbg_gout", [128, int(L_sc[0]) * 4],
                                  dt.bfloat16, kind="ExternalOutput").ap()
        dbg_msg = nc.dram_tensor("dbg_msg", [128, 512], dt.bfloat16,
                                 kind="ExternalOutput").ap()
        dbg_agg = nc.dram_tensor("dbg_agg", [W, G_WIN * D], dt.float32,
                                 kind="ExternalOutput").ap()

    with tile.TileContext(nc) as tc:
        with (
            tc.tile_pool(name="dram", bufs=1, space="DRAM") as dram,
            tc.tile_pool(name="const", bufs=1) as const,
            tc.tile_pool(name="tabp", bufs=1) as tabp,
            tc.tile_pool(name="goutp", bufs=2) as goutp,
            tc.tile_pool(name="msgp", bufs=3) as msgpool,
            tc.tile_pool(name="ohp", bufs=3) as ohp,
            tc.tile_pool(name="psm", bufs=2, space="PSUM") as psm,
            tc.tile_pool(name="psa", bufs=4, space="PSUM") as psa,
            tc.tile_pool(name="pst", bufs=2, space="PSUM") as pst,
            tc.tile_pool(name="resp", bufs=2) as resp,
            tc.tile_pool(name="stgp", bufs=2) as stgpool,
        ):
            c_split = agsplit * G_WIN * 512
            tin_a = dram.tile([16, c_split], dt.bfloat16, tag="tina", name="tin_a")
            tin_b = dram.tile([16, NT - c_split], dt.bfloat16, tag="tinb",
                              name="tin_b")
            tt_a = dram.tile([128, c_split], dt.bfloat16, tag="tta", name="tt_a")
            tt_b = dram.tile([128, NT - c_split], dt.bfloat16, tag="ttb",
                             name="tt_b")

            idx_t = const.tile_from(idx_d)
            dstw_t = const.tile_from(dstw_d)
            io_t = const.tile_from(iota_d)
            id_t = const.tile_from(ident_d)
            a_t = const.tile_from(a_d)
            b_t = const.tile_from(b_d)
            a2_t = const.tile_from(a2_d)
            b2_t = const.tile_from(b2_d)

            table = tabp.tile([128, NT], dt.bfloat16, tag="table", name="table")
            # initial table load
            nc.sync.dma_start(out=table[:, :], in_=tt0[:, :])

            for step in range(steps):
                last = step == steps - 1
                if step > 0:
                    # AllGather halves of Tin (written by previous step)
                    for (ti, to) in ((tin_a, tt_a), (tin_b, tt_b)):
                        nc.gpsimd.collective_compute(
                            "AllGather",
                            mybir.AluOpType.bypass,
                            replica_groups=[list(range(M))],
                            ins=[ti[:, :].opt()],
                            outs=[to[:, :].opt()],
                        )
                    nc.sync.dma_start(out=table[:, :c_split], in_=tt_a[:, :])
                    nc.sync.dma_start(out=table[:, c_split:], in_=tt_b[:, :])

                for s in range(NSC):
                    Ls = int(L_sc[s])
                    gout = goutp.tile([128, Lmax * 4], dt.bfloat16, tag="gout",
                                      name="gout")
                    nc.gpsimd.ap_gather(
                        out_ap=gout[:, :Ls * 4],
                        in_ap=table[:, :],
                        idxs_ap=idx_t[:, L_off[s] // 16:L_off[s + 1] // 16],
                        channels=128, num_elems=SHARD, d=4, num_idxs=Ls,
                    )
                    gout3 = gout[:, :Ls * 4].rearrange("p (e r) -> p e r", r=4)
                    if dbg and step == 0 and s == 0:
                        nc.sync.dma_start(out=dbg_gout[:, :], in_=gout[:, :Ls * 4])

                    stg = None
                    if not last:
                        stg = stgpool.tile([16, G_WIN * 512], dt.bfloat16,
                                           tag="stg", name="stg")
                    else:
                        o32 = resp.tile([W, G_WIN, D], dt.float32, tag="o32",
                                        name="o32")

                    aggs = {}
                    for (k0, k1, entry) in segs[s]:
                        nk = k1 - k0
                        msgps = psm.tile([128, 512], dt.float32, tag="msgps",
                                         name="msgps")
                        for r in range(4):
                            nc.tensor.matmul(
                                out=msgps[0:nk, 128 * r:128 * r + 128],
                                lhsT=gout3[:, k0:k1, r],
                                rhs=id_t[:, :],
                                start=True, stop=True,
                            )
                        msg = msgpool.tile([128, 512], dt.bfloat16, tag="msg",
                                           name="msg")
                        nc.vector.tensor_copy(out=msg[0:nk, :], in_=msgps[0:nk, :])
                        msg3 = msg[:, :].rearrange("p (r gj) -> p r gj", gj=128)
                        if dbg and step == 0 and s == 0 and k0 == 0:
                            nc.sync.dma_start(out=dbg_msg[:, :], in_=msg[:, :])

                        ns_k = len(entry)
                        c0 = entry[0][2]
                        oh = ohp.tile([128, ns_k, W], dt.bfloat16,
                                      tag="oh", name="oh")
                        nc.vector.tensor_tensor(
                            out=oh[:, :ns_k, :],
                            in0=dstw_t[:, c0:c0 + ns_k, None]
                                .to_broadcast([128, ns_k, W]),
                            in1=io_t[:, None, :].to_broadcast([128, ns_k, W]),
                            op=mybir.AluOpType.is_equal,
                        )
                        for (gg, wglob, colx, _a, _b) in entry:
                            wi = wglob - s * G_WIN
                            if startf[colx]:
                                aggs[wi] = psa.tile([W, D], dt.float32,
                                                    tag="agg", name="agg")
                            nc.tensor.matmul(
                                out=aggs[wi][:, :],
                                lhsT=oh[0:nk, colx - c0, :],
                                rhs=msg3[0:nk, :, 16 * gg:16 * gg + 16],
                                start=bool(startf[colx]),
                                stop=bool(stopf[colx]),
                            )
                            if stopf[colx]:
                                wg = wglob
                                if dbg and step == 0 and s == 0:
                                    d32 = resp.tile([W, D], dt.float32,
                                                    tag="d32", name="d32")
                                    nc.vector.tensor_copy(out=d32[:, :],
                                                          in_=aggs[wi][:, :])
                                    nc.sync.dma_start(
                                        out=dbg_agg[:, wi * D:(wi + 1) * D],
                                        in_=d32[:, :])
                                ca = (a2_t if last else a_t)[:, wg:wg + 1]
                                cb = (b2_t if last else b_t)[:, wg * D:(wg + 1) * D]
                                tmp = resp.tile([W, D], dt.float32, tag="tmp",
                                                name="tmp")
                                nc.vector.tensor_scalar(
                                    out=tmp[:, :], in0=aggs[wi][:, :],
                                    scalar1=ca, scalar2=None,
                                    op0=mybir.AluOpType.mult,
                                )
                                if last:
                                    nc.vector.tensor_tensor(
                                        out=o32[:, wi, :], in0=tmp[:, :], in1=cb,
                                        op=mybir.AluOpType.add,
                                    )
                                else:
                                    gwb = resp.tile([W, D], dt.bfloat16,
                                                    tag="gwb", name="gwb")
                                    nc.vector.tensor_tensor(
                                        out=gwb[:, :], in0=tmp[:, :], in1=cb,
                                        op=mybir.AluOpType.add,
                                    )
                                    stgps = pst.tile([16, 512], dt.float32,
                                                     tag="stgps", name="stgps")
                                    st3 = stgps[:, :].rearrange(
                                        "p (r mm) -> p r mm", r=4)
                                    for r in range(4):
                                        nc.tensor.matmul(
                                            out=st3[:, r, :],
                                            lhsT=gwb[:, 16 * r:16 * r + 16],
                                            rhs=id_t[:, :],
                                            start=True, stop=True,
                                        )
                                    nc.vector.tensor_copy(
                                        out=stg[:, 512 * wi:512 * (wi + 1)]
                                            .rearrange("p (mm r) -> p r mm", r=4),
                                        in_=st3[:, :, :],
                                    )
                    if last:
                        w0 = s * G_WIN
                        nc.sync.dma_start(
                            out=out[w0 * W:(w0 + G_WIN) * W, :]
                                .rearrange("(a p) d -> p a d", p=W),
                            in_=o32[:, :, :],
                        )
                    else:
                        if s < agsplit:
                            nc.sync.dma_start(
                                out=tin_a[:, s * G_WIN * 512:(s + 1) * G_WIN * 512],
                                in_=stg[:, :],
                            )
                        else:
                            sb = s - agsplit
                            nc.sync.dma_start(
                                out=tin_b[:, sb * G_WIN * 512:(sb + 1) * G_WIN * 512],
                                in_=stg[:, :],
                            )
    nc.compile()
    return nc


def _make_inputs(features, src, dst, meta, M, D, alpha):
    n = features.shape[0]
    npad = SHARD * M

    deg_out = np.bincount(src, minlength=n).astype(np.float32)
    deg_in = np.bincount(dst, minlength=n).astype(np.float32)
    ns = np.clip(deg_out, 1.0, None) ** -0.5
    nd = np.clip(deg_in, 1.0, None) ** -0.5

    ns_pad = np.ones(npad, dtype=np.float32)
    nd_pad = np.ones(npad, dtype=np.float32)
    h0_pad = np.zeros((npad, D), dtype=np.float32)
    ns_pad[:n] = ns
    nd_pad[:n] = nd
    h0_pad[:n] = features

    g0 = (h0_pad * ns_pad[:, None]).astype(BF16)      # [npad, 64]
    # transposed quad table: row 16g+j, col 4m+r = g0[g*SHARD+m, 16r+j]
    tt0 = np.zeros((128, SHARD * 4), dtype=BF16)
    for g in range(M):
        blk = g0[g * SHARD:(g + 1) * SHARD]           # [SHARD, 64]
        b4 = blk.reshape(SHARD, 4, 16)                # [m, r, j]
        tt0[16 * g:16 * g + 16] = b4.transpose(2, 0, 1).reshape(16, SHARD * 4)

    iota = np.tile(np.arange(W, dtype=np.float32), (128, 1)).astype(BF16)
    ident = np.eye(128, dtype=np.float32).astype(BF16)

    in_maps = []
    for c in range(M):
        sl = slice(c * SHARD, (c + 1) * SHARD)
        a = ((1.0 - alpha) * nd_pad[sl] * ns_pad[sl]).astype(np.float32)
        b = (alpha * h0_pad[sl] * ns_pad[sl][:, None]).astype(np.float32)
        a2 = ((1.0 - alpha) * nd_pad[sl]).astype(np.float32)
        b2 = (alpha * h0_pad[sl]).astype(np.float32)

        def wmaj(x):
            x = x.reshape(NW, W, -1).transpose(1, 0, 2)
            return np.ascontiguousarray(x.reshape(W, -1))

        in_maps.append({
            "tt0": tt0,
            "idx": meta["idx16"][c],
            "dstw": meta["dstw"][c],
            "iota": iota,
            "ident": ident,
            "acoef": wmaj(a).astype(np.float32),
            "bcoef": wmaj(b).astype(BF16),
            "acoef2": wmaj(a2).astype(np.float32),
            "bcoef2": wmaj(b2).astype(BF16),
        })
    return in_maps


_NC_CACHE = {}


def build_all(features, src, dst, *, n_nodes=None, M=M_CORES, D=D_FEAT,
              steps=K_LAYERS, alpha=ALPHA, agsplit=None, **kw):
    import os
    if agsplit is None:
        agsplit = int(os.environ.get("APPNP_AGSPLIT", "10"))
    src = np.asarray(src).astype(np.int64)
    dst = np.asarray(dst).astype(np.int64)
    meta = _preprocess(src, dst, M)
    key = (meta["L_tot"], meta["nseg_tot"], steps, M, agsplit)
    if key not in _NC_CACHE:
        _NC_CACHE[key] = _build_nc(meta, M, D, steps, agsplit)
    nc = _NC_CACHE[key]
    in_maps = _make_inputs(np.asarray(features, np.float32), src, dst, meta, M, D,
                           alpha)
    return nc, in_maps, meta


def kernel(features, src, dst, *, trace=False, **kw):
    from concourse.bass_utils import run_bass_kernel_spmd

    features = np.asarray(features)
    nc, in_maps, meta = build_all(features, src, dst, **kw)
    res = run_bass_kernel_spmd(nc, in_maps, core_ids=list(range(M_CORES)),
                               trace=trace)
    n = features.shape[0]
    h = np.concatenate([res.results[c]["out"] for c in range(M_CORES)], axis=0)
    out = np.ascontiguousarray(h[:n]).astype(np.float32)
    if trace:
        kernel.last_results = res
    return out

